# revision 26
# baseline (speedup 1.0000x reference)
"""Trainium2 Bass kernel for session-GNN attention readout (8 NeuronCores).

Math per stream (feats = feats_invar or feats_var):
  u = feats @ Wu + bu                    [N, H]
  v = (feats[last_nodes] @ Wv)[seg_ids]  [N, H]
  e = sigmoid(u + v) @ We                [N, 1]
  a = segment_softmax(e)                 [N, 1]
  out = segment_sum(feats * a)           [B, D]

Sharding: nodes across 8 cores, whole segments per core (1024 segments =
65536 nodes per core). Params replicated. No cross-core communication.

Device pipeline per core (heavy data bf16 on-chip):
  - SWDGE cast-DMA loads feats fp32 -> SBUF bf16 natural (1 MB blocks)
  - HWDGE xbar DMA-transpose -> feats^T [d, node] (32 blocks per op)
  - PE: z^T = Wu^T @ feats^T  accumulated with the segment-broadcast of
    v (selector-matrix matmul) in the same PSUM banks
  - ACT: x^T = sigmoid(z^T + bu)  (bias = per-partition bu column)
  - PE: e packed row-wise into one PSUM bank via sliding-window We
    placement (matmul r writes only row r of the bank, rows accumulate)
  - sigmoid via tanh: sigma(z) = (1+tanh(z/2))/2, and the constant
    shift cancels in softmax -> a = softmax(0.5 * e_tanh). tanh and exp
    live in the same ACT table set (no table reloads).
  - softmax without max-subtraction (|e| <= sum|We| ~ 11 -> exp safe in
    fp32; ratios identical): ACT exp(0.5x), DVE grouped segment sums +
    reciprocal, A = ex/den (bf16)
  - A -> node-partition layout via DVE 32x32 block transpose + 4 masked
    expansions into a block-diagonal [128, tiles, 2] moving operand
  - PE readout: per 128-node tile, stationary = natural feats tile,
    moving = A_blk[:, t, :] (N=2) -> out^T[d, 2 segs] written to PSUM
    free slots; one DVE evac per stage -> out^T [d, seg] bf16; host
    transposes after gather. Readout matmuls are interleaved into the
    next stage's PE stream (software pipelining by emission order).
  - loads and xbar transposes are pair-grouped: Tile serializes xbar
    transposes against all other DMA traffic (HW xbar-mode bug guard),
    so fewer mode transitions = less serial DMA time.
"""

import numpy as np

N = 524288
B = 8192
AVG = 64
D = 128
H = 128
NCORES = 8

NL = N // NCORES          # 65536 nodes per core
BL = B // NCORES          # 1024 segments per core
KC = 1024                 # nodes per kchunk (PSUM-sized compute unit)
NKC = NL // KC            # 64 kchunks per stream
BLK = 4                   # kchunks per DMA/readout block (4096 nodes)
KB = BLK * KC             # 4096
QK = 8                    # kchunks per pipeline stage ("quarter")
NQ = NKC // QK            # stages per stream
QN = QK * KC              # 8192 nodes per stage

_CACHE = {}


def _build():
    from concourse import bacc, mybir
    from concourse.tile import TileContext

    bf16 = mybir.dt.bfloat16
    f32 = mybir.dt.float32
    AF = mybir.ActivationFunctionType

    nc = bacc.Bacc(None, target_bir_lowering=False)

    f_in = {}
    out_p = {}
    for s in ("i", "v"):
        f_in[s] = nc.declare_dram_parameter(f"feats_{s}", [NL, D], f32, isOutput=False)
    Wu = nc.declare_dram_parameter("Wu", [D, H], f32, isOutput=False)
    bu = nc.declare_dram_parameter("bu", [H], f32, isOutput=False)
    Wv = nc.declare_dram_parameter("Wv", [D, H], f32, isOutput=False)
    We = nc.declare_dram_parameter("We", [H, 1], f32, isOutput=False)
    Sm = nc.declare_dram_parameter("Sm", [128, KC], bf16, isOutput=False)
    Wz = nc.declare_dram_parameter("Wz", [H, 255], bf16, isOutput=False)
    Mask2 = nc.declare_dram_parameter("Mask2", [32, 4, 2], bf16, isOutput=False)
    for s in ("i", "v"):
        # transposed output [d, seg]; host transposes back
        out_p[s] = nc.declare_dram_parameter(f"out_{s}", [D, BL], f32, isOutput=True)

    with TileContext(nc) as tc:
        with (
            tc.tile_pool(name="const", bufs=1) as cpool,
            tc.tile_pool(name="bigf", bufs=4) as bigpool,
            tc.tile_pool(name="bigt", bufs=3) as bigtpool,
            tc.tile_pool(name="work", bufs=3) as wpool,
            tc.tile_pool(name="xw", bufs=3) as xwpool,
            tc.tile_pool(name="ro", bufs=2) as ropool,
            tc.tile_pool(name="mo", bufs=3) as mopool,
            tc.tile_pool(name="soft", bufs=2) as spool,
            tc.tile_pool(name="flat", bufs=2) as fpool,
        ):
            # ---------- constants ----------
            wu_t = cpool.tile([D, H], bf16, tag="wu")
            nc.gpsimd.dma_start(out=wu_t[:], in_=Wu[:])
            wv_t = cpool.tile([D, H], bf16, tag="wv")
            nc.gpsimd.dma_start(out=wv_t[:], in_=Wv[:])
            bu_t = cpool.tile([H, 1], f32, tag="bu")
            nc.sync.dma_start(out=bu_t[:], in_=bu[:].rearrange("(h o) -> h o", o=1))

            wz = cpool.tile([H, 255], bf16, tag="wz")
            nc.sync.dma_start(out=wz[:], in_=Wz[:])
            S = cpool.tile([128, KC], bf16, tag="S")
            nc.sync.dma_start(out=S[:], in_=Sm[:])
            mask2 = cpool.tile([32, 4, 2], bf16, tag="mask2")
            nc.sync.dma_start(out=mask2[:], in_=Mask2[:])

            # persistent transposed outputs [d, seg] (bf16; cast on store)
            outT = {
                s: cpool.tile([D, BL], bf16, tag=f"outT{s}", name=f"outT{s}")
                for s in ("i", "v")
            }

            # ---------- pre-phase: v = feats[last] @ Wv (seg-major bf16) ----
            # v_seg[s][j, c*H + h] = v value for segment 16c + j, channel h
            v_seg = {
                s: cpool.tile([48, NKC * H], bf16, tag=f"vseg{s}", name=f"vseg{s}")
                for s in ("i", "v")
            }
            with (
                tc.tile_pool(name="vpre", bufs=2) as vpool,
                tc.tile_pool(name="vpsum", bufs=2, space="PSUM") as vppool,
            ):
                for s in ("i", "v"):
                    fl_nat = vpool.tile([128, BL // 128, D], bf16, tag="flnat")
                    nc.gpsimd.dma_start(
                        out=fl_nat[:, :, None, :],
                        in_=f_in[s][:].rearrange(
                            "(g p a) d -> p g a d", p=128, a=AVG
                        )[:, :, AVG - 1 : AVG, :],
                    )
                    flT = vpool.tile([128, BL // 128, 128], bf16, tag="flT")
                    nc.sync.dma_start_transpose(out=flT[:], in_=fl_nat[:])
                    for c in range(NKC):  # 16 segments per chunk
                        g, off = divmod(16 * c, 128)
                        vp = vppool.tile([16, 512], f32, tag="vp")
                        nc.tensor.matmul(
                            out=vp[0:16, 0:H],
                            lhsT=flT[:, g, off : off + 16],
                            rhs=wv_t[:],
                            start=True,
                            stop=True,
                        )
                        nc.vector.tensor_copy(
                            v_seg[s][0:16, c * H : (c + 1) * H], vp[0:16, 0:H]
                        )
                    nc.sync.dma_start(
                        out=v_seg[s][32:48, :], in_=v_seg[s][0:16, :]
                    )

            # ---------- main: software-pipelined stages ----------
            TQ = QN // 128          # node tiles per stage
            RO_CKS = 4
            RO_SLOTS = [TQ * i // RO_CKS for i in range(RO_CKS + 1)]

            with (
                tc.tile_pool(name="zpsum", bufs=2, space="PSUM") as zpool,
                tc.tile_pool(name="epsum", bufs=2, space="PSUM") as epool,
                tc.tile_pool(name="ropsum", bufs=2, space="PSUM") as rpool,
            ):
                def _emit_ro_tiles(ro, t0, t1):
                    _s, _q, _fnat, _A_blk, _outPS = ro
                    for t in range(t0, t1):
                        nc.tensor.matmul(
                            out=_outPS[:, 2 * t : 2 * t + 2],
                            lhsT=_fnat[:, t, :],
                            rhs=_A_blk[:, t, :],
                            start=True,
                            stop=True,
                            skip_group_check=True,
                        )

                def _evac_ro(ro):
                    _s, _q, _fnat, _A_blk, _outPS = ro
                    segs = QN // AVG
                    nc.vector.tensor_copy(
                        outT[_s][:, _q * segs : (_q + 1) * segs], _outPS[:]
                    )

                def _emit_ablk(pend):
                    # DVE 32x32 block-transpose of Ap (padded to 32 rows),
                    # then 4 per-partition-group masked expansions.
                    # AT2[j, 128*fb + 32*c + r] = Ap[r, 128*fb + 32*c + j]
                    _s, _q, _fnat, _Ap = pend
                    AT2 = ropool.tile([32, 512], bf16, tag="AT2",
                                      name=f"AT2_{_s}{_q}")
                    nc.vector.transpose(out=AT2[:], in_=_Ap[:])
                    A_blk = ropool.tile([128, TQ, 2], bf16, tag="A_blk",
                                        name=f"A_blk_{_s}{_q}")
                    for c in range(4):
                        nc.vector.tensor_tensor(
                            out=A_blk[32 * c : 32 * (c + 1), :, :].rearrange(
                                "p (r fb) cc -> p r fb cc", fb=4
                            ),
                            in0=AT2[:]
                            .rearrange("p (fb cc r) -> p fb cc r", fb=4, cc=4)[
                                :, :, c, 0 : 2 * QK
                            ]
                            .rearrange("p fb r -> p r fb")[:, :, :, None]
                            .broadcast_to([32, 2 * QK, 4, 2]),
                            in1=mask2[:, c, :][:, None, None, :]
                            .broadcast_to([32, 2 * QK, 4, 2]),
                            op=mybir.AluOpType.mult,
                        )
                    outPS = rpool.tile([128, 2 * TQ], f32, tag="outPS",
                                       name=f"outPS_{_s}{_q}")
                    return (_s, _q, _fnat, A_blk, outPS)

                pending = None      # stage with softmax phase-1 done
                ro = None           # stage being read out
                stages = [(s, q) for s in ("i", "v") for q in range(NQ)]
                stage_io = {}
                from concourse.tile_rust import add_dep_helper

                for p0 in range(0, len(stages), 2):
                    grp = stages[p0 : p0 + 2]
                    load_insts = []
                    for s, q in grp:
                        qbase = q * QN
                        fnat = bigpool.tile([128, TQ, D], bf16, tag="fnat",
                                            name=f"fnat_{s}{q}")
                        li = nc.gpsimd.dma_start(
                            out=fnat[:],
                            in_=f_in[s][qbase : qbase + QN, :].rearrange(
                                "(g p) d -> p g d", p=128
                            ),
                        )
                        load_insts.append(li)
                        stage_io[(s, q)] = [fnat, None]
                    for s, q in grp:
                        fT = bigtpool.tile([128, QN], bf16, tag="fT",
                                           name=f"fT_{s}{q}")
                        ti = nc.sync.dma_start_transpose(
                            out=fT[:].rearrange("p (b n) -> p b n", n=128),
                            in_=stage_io[(s, q)][0],
                        )
                        # force both group loads before either transpose so
                        # the xbar-mode serialization chain groups [L,L][T,T]
                        for li in load_insts:
                            add_dep_helper(ti.ins, li.ins,
                                           reason="group loads before transposes")
                        stage_io[(s, q)][1] = fT
                    for s, q in grp:
                        qbase = q * QN
                        fnat, fT = stage_io.pop((s, q))
                        ebank = epool.tile([128, 512], f32, tag="ebank")
                        if pending is not None:
                            ro = _emit_ablk(pending)
                            pending = None
                        for ck in range(QK):
                            cg = q * QK + ck
                            fTc = fT[:, ck * KC : (ck + 1) * KC]
                            zT = zpool.tile([128, KC], f32, tag="zT")
                            xT = xwpool.tile([128, KC], bf16, tag="xT")
                            for h in range(2):
                                cols = slice(h * 512, (h + 1) * 512)
                                nc.tensor.matmul(
                                    out=zT[:, cols],
                                    lhsT=wu_t[:],
                                    rhs=fTc[:, cols],
                                    start=True,
                                    stop=False,
                                    skip_group_check=True,
                                )
                            # vb halves packed into disjoint PE row groups
                            for h in range(2):
                                cols = slice(h * 512, (h + 1) * 512)
                                base = 32 * h
                                nc.tensor.matmul(
                                    out=zT[:, cols],
                                    lhsT=v_seg[s][
                                        base : base + 16, cg * H : (cg + 1) * H
                                    ],
                                    rhs=S[base : base + 16, cols],
                                    start=False,
                                    stop=True,
                                    tile_position=(base, 0),
                                    skip_group_check=True,
                                )
                            nc.scalar.activation(
                                out=xT[:], in_=zT[:], func=AF.Tanh,
                                bias=bu_t[:], scale=0.5,
                            )
                            for h in range(2):
                                r = 2 * ck + h
                                nc.tensor.matmul(
                                    out=ebank[:],
                                    lhsT=wz[:, 127 - r : 255 - r],
                                    rhs=xT[:, h * 512 : (h + 1) * 512],
                                    start=(r == 0),
                                    stop=(r == 2 * QK - 1),
                                    skip_group_check=True,
                                )
                            if ro is not None and 1 <= ck <= RO_CKS:
                                _emit_ro_tiles(ro, RO_SLOTS[ck - 1], RO_SLOTS[ck])
                        if ro is not None:
                            _evac_ro(ro)
                            ro = None
                        # softmax phase 1 (exp / denom / recip / A)
                        ex = spool.tile([2 * QK, 512], f32, tag="ex")
                        nc.scalar.activation(
                            out=ex[:], in_=ebank[0 : 2 * QK, :], func=AF.Exp,
                            scale=0.5,
                        )
                        den = spool.tile([2 * QK, 8], f32, tag="den")
                        nc.vector.reduce_sum(
                            out=den[:],
                            in_=ex[:].rearrange("p (g a) -> p g a", a=AVG),
                            axis=mybir.AxisListType.X,
                        )
                        rden = spool.tile([2 * QK, 8], f32, tag="rden")
                        nc.vector.reciprocal(out=rden[:], in_=den[:])
                        Ap = spool.tile([32, 512], bf16, tag="Ap")
                        nc.vector.tensor_tensor(
                            out=Ap[0 : 2 * QK, :].rearrange("p (g a) -> p g a", a=AVG),
                            in0=ex[:].rearrange("p (g a) -> p g a", a=AVG),
                            in1=rden[:, :, None].broadcast_to([2 * QK, 8, AVG]),
                            op=mybir.AluOpType.mult,
                        )
                        pending = (s, q, fnat, Ap)
                # drain the pipeline tail
                ro = _emit_ablk(pending)
                _emit_ro_tiles(ro, 0, TQ)
                _evac_ro(ro)
                for s in ("i", "v"):
                    nc.gpsimd.dma_start(out=out_p[s][:], in_=outT[s][:])

    nc.finalize()
    return nc


def _get_nc():
    if "nc" not in _CACHE:
        _CACHE["nc"] = _build()
    return _CACHE["nc"]


def make_in_maps(feats_invar, feats_var, Wu, bu, Wv, We):
    import ml_dtypes

    Sm = np.zeros((128, KC), dtype=ml_dtypes.bfloat16)
    for j in range(16):
        Sm[j, j * AVG : (j + 1) * AVG] = 1.0
        Sm[32 + j, j * AVG : (j + 1) * AVG] = 1.0
    Wz = np.zeros((H, 255), dtype=ml_dtypes.bfloat16)
    Wz[:, 127] = We[:, 0].astype(ml_dtypes.bfloat16)
    Mask2 = np.zeros((32, 4, 2), dtype=ml_dtypes.bfloat16)
    Mask2[:, :2, 0] = 1.0
    Mask2[:, 2:, 1] = 1.0
    in_maps = []
    for c in range(NCORES):
        sl = slice(c * NL, (c + 1) * NL)
        in_maps.append(
            {
                "feats_i": feats_invar[sl],
                "feats_v": feats_var[sl],
                "Wu": Wu,
                "bu": (0.5 * bu).astype(np.float32),
                "Wv": Wv,
                "We": We,
                "Sm": Sm,
                "Wz": Wz,
                "Mask2": Mask2,
            }
        )
    return in_maps


def _reference_numpy(feats_invar, feats_var, Wu, bu, Wv, We, seg_ids, last_nodes):
    """Generic fallback (never used for the uniform-segment inputs)."""
    num_seg = last_nodes.shape[0]
    outs = []
    for f in (feats_invar, feats_var):
        u = f @ Wu + bu
        v = (f[last_nodes] @ Wv)[seg_ids]
        e = (1.0 / (1.0 + np.exp(-(u + v)))) @ We
        mx = np.full((num_seg, 1), -np.inf, np.float32)
        np.maximum.at(mx, seg_ids, e)
        ex = np.exp(e - mx[seg_ids])
        dn = np.zeros((num_seg, 1), np.float32)
        np.add.at(dn, seg_ids, ex)
        a = ex / dn[seg_ids]
        r = np.zeros((num_seg, f.shape[1]), np.float32)
        np.add.at(r, seg_ids, f * a)
        outs.append(r[:, None, :])
    return tuple(outs)


def kernel(**inputs):
    feats_invar = np.ascontiguousarray(inputs["feats_invar"], dtype=np.float32)
    feats_var = np.ascontiguousarray(inputs["feats_var"], dtype=np.float32)
    Wu = np.ascontiguousarray(inputs["Wu"], dtype=np.float32)
    bu = np.ascontiguousarray(inputs["bu"], dtype=np.float32)
    Wv = np.ascontiguousarray(inputs["Wv"], dtype=np.float32)
    We = np.ascontiguousarray(inputs["We"], dtype=np.float32)
    seg_ids = np.asarray(inputs["seg_ids"])
    last_nodes = np.asarray(inputs["last_nodes"])

    uniform = (
        feats_invar.shape == (N, D)
        and np.array_equal(seg_ids, (np.arange(N, dtype=np.int64) // AVG))
        and np.array_equal(last_nodes, np.arange(B, dtype=np.int64) * AVG + AVG - 1)
    )
    if not uniform:
        return _reference_numpy(
            feats_invar, feats_var, Wu, bu, Wv, We, seg_ids, last_nodes
        )

    from concourse.bass_utils import run_bass_kernel_spmd

    nc = _get_nc()
    in_maps = make_in_maps(feats_invar, feats_var, Wu, bu, Wv, We)
    res = run_bass_kernel_spmd(nc, in_maps, core_ids=list(range(NCORES)))
    rst_i = np.concatenate(
        [np.ascontiguousarray(res.results[c]["out_i"].T) for c in range(NCORES)], axis=0
    )[:, None, :]
    rst_v = np.concatenate(
        [np.ascontiguousarray(res.results[c]["out_v"].T) for c in range(NCORES)], axis=0
    )[:, None, :]
    return rst_i, rst_v
